# revision 25
# baseline (speedup 1.0000x reference)
"""Trainium2 Bass kernel for nn_AttentionModel (sparse_attention).

Strategy: pure data parallel over 8 NeuronCores (128 samples each), weights
replicated. BatchNorm statistics are made *exact* (matching single-device
reference) via a tiny AllReduce of per-core [sum, sumsq] per channel.

Device layout: activations live transposed as hT [D=128 partitions,
positions = sample*131+node]. Per-sample attention (131x131, single head,
dim 128) is computed with PE matmuls; key masking is applied with a rank-1
accumulating matmul (ones^T @ mask_row) so no partition-broadcast is needed.
Softmax skips max-subtraction (scores are bounded: |s| < 13 measured, and
masked entries are -1e9 -> exp underflows to exactly 0).

Final probs + ctx are computed on device; alp/selected/entropy are cheap
[B,50] reductions done on host in fp32.

fp32 everywhere: top-2 prob gaps go down to 2.7e-4, so bf16/fp32r matmuls
would flip argmax outputs.
"""

import sys

sys.path.insert(0, "/opt/trn_rl_repo")

import numpy as np
import ml_dtypes

import concourse.bass as bass
import concourse.tile as tile
from concourse import bacc
from concourse import mybir
from concourse.alu_op_type import AluOpType
from concourse.bass_utils import run_bass_kernel_spmd

F32 = mybir.dt.float32
BF16 = mybir.dt.bfloat16
AF = mybir.ActivationFunctionType
AX = mybir.AxisListType

NCORES = 8
B = 1024
BSH = B // NCORES  # 128 samples per core
IH, LH, NN = 80, 50, 131
D = 128
FFH = 512
NPOS = BSH * NN  # 16768
NTOT = B * NN  # 134144 positions globally
ISD = 1.0 / np.sqrt(np.float32(D))

# ---- mega weight-tile column layout (fp32 [128, MEGA_COLS]) ----
_c = 0
def _alloc(n):
    global _c
    off = _c
    _c += n
    return off

IDC = _alloc(128)
WQ = [_alloc(128) for _ in range(2)]
WK = [_alloc(128) for _ in range(2)]
WV = [_alloc(128) for _ in range(2)]
WO = [_alloc(128) for _ in range(2)]
FW1 = [_alloc(512) for _ in range(2)]
FW2P = [_alloc(512) for _ in range(2)]  # packed: chunk c at +c*128 = ffw2[c*128:(c+1)*128, :]
WCTX = _alloc(128)
WNODE = _alloc(128)  # first D cols of w_node, scaled by 1/sqrt(D)
W2INT = _alloc(128)  # rows 0:32
W2LEAF = _alloc(128)
W2NEXT = _alloc(128)
W1INT = _alloc(32)  # rows 0:6
W1LEAF = _alloc(32)  # rows 0:8
W1NEXT = _alloc(32)  # rows 0:6
B1INT = _alloc(1)  # rows 0:32
B1LEAF = _alloc(1)
B1NEXT = _alloc(1)
B2INT = _alloc(1)  # rows 0:128
B2LEAF = _alloc(1)
B2NEXT = _alloc(1)
FB1 = [_alloc(4) for _ in range(2)]  # chunk c at +c, rows = ffb1[c*128:(c+1)*128]
N1G = [_alloc(1) for _ in range(2)]
N1B = [_alloc(1) for _ in range(2)]
N2G = [_alloc(1) for _ in range(2)]
N2B = [_alloc(1) for _ in range(2)]
INVLEN = _alloc(1)  # per-core 1/valid_length, [128,1]
LEAFM = _alloc(LH)  # per-core leaf valid mask (0/1) [128,50]
MEGA_COLS = ((_c + 31) // 32) * 32


def _sample_blocks(bs, nper):
    out = []
    s0 = 0
    while s0 < bs:
        out.append((s0, min(nper, bs - s0)))
        s0 += nper
    return out


def _col_blocks(total, w):
    out = []
    c0 = 0
    while c0 < total:
        out.append((c0, min(w, total - c0)))
        c0 += w
    return out


def build_bass(ncores=NCORES):
    nc = bacc.Bacc("TRN2", num_devices=ncores, target_bir_lowering=False)

    mega = nc.dram_tensor("mega", [128, MEGA_COLS], F32, kind="ExternalInput")
    xint = nc.dram_tensor("xint", [6, BSH * IH], F32, kind="ExternalInput")
    xleaf = nc.dram_tensor("xleaf", [8, BSH * LH], F32, kind="ExternalInput")
    xnext = nc.dram_tensor("xnext", [6, BSH], F32, kind="ExternalInput")
    masksbf = nc.dram_tensor("masksbf", [BSH, NN], BF16, kind="ExternalInput")
    validbf = nc.dram_tensor("validbf", [1, NPOS], BF16, kind="ExternalInput")
    onesbf = nc.dram_tensor("onesbf", [1, 128], BF16, kind="ExternalInput")
    OutComp = nc.dram_tensor("OutComp", [1, BSH * LH], F32, kind="ExternalOutput")
    OutCtx = nc.dram_tensor("OutCtx", [BSH, D], F32, kind="ExternalOutput")

    with tile.TileContext(nc) as tc:
        with tc.tile_pool(name="persist", bufs=1) as persist, \
             tc.tile_pool(name="drampool", bufs=1, space="DRAM") as drampool:
            mg = persist.tile([128, MEGA_COLS], F32)
            nc.sync.dma_start(out=mg, in_=mega[:])
            ob = persist.tile([1, 128], BF16)
            nc.sync.dma_start(out=ob, in_=onesbf[:])
            ident = mg[:, IDC:IDC + 128]

            # PE fences: absorb the mega/ones DMA waits once so later matmuls
            # carry at most one semaphore wait (fp32 matmul HW limit).
            with tc.tile_pool(name="fencep", bufs=1, space="PSUM") as fencep:
                f1 = fencep.tile([1, 128], F32)
                nc.tensor.transpose(f1[:], mg[:, IDC:IDC + 1], ident)
                f2 = fencep.tile([1, 1], F32)
                nc.tensor.matmul(f2[:], ob[0:1, 0:1], ob[0:1, 0:1], start=True, stop=True)

            hT = persist.tile([128, NPOS], F32)
            hT3 = hT[:].rearrange("p (s n) -> p s n", n=NN)
            hTv = hT[:].rearrange("p (g c) -> p g c", c=262)
            bn_stats_tiles = []
            for i in range(4):
                bnst = persist.tile([128, 64, 6], F32, tag=f"bnst{i}")
                bn_stats_tiles.append(bnst)

            def emit_stats(bn_idx, emitted, upto_col):
                # emit bn_stats for every 262-col group fully covered by
                # residuals completed up to upto_col (pipelines the BN
                # reduction with the preceding residual loop)
                stats = bn_stats_tiles[bn_idx]
                while emitted[0] < 64 and 262 * (emitted[0] + 1) <= upto_col:
                    g = emitted[0]
                    nc.vector.bn_stats(out=stats[:, g, :], in_=hTv[:, g, :])
                    emitted[0] += 1

            # ---------------- init embeddings ----------------
            with tc.tile_pool(name="initin", bufs=1) as iin, \
                 tc.tile_pool(name="initsb", bufs=2) as isb, \
                 tc.tile_pool(name="initps", bufs=2, space="PSUM") as ips:
                xi = iin.tile([6, BSH * IH], F32, tag="xin_i")
                nc.sync.dma_start(out=xi, in_=xint[:])
                xl = iin.tile([8, BSH * LH], F32, tag="xin_l")
                nc.sync.dma_start(out=xl, in_=xleaf[:])
                xn = iin.tile([6, BSH], F32, tag="xin_n")
                nc.sync.dma_start(out=xn, in_=xnext[:])

                def mlp_phase(xtile, nf, per, w1c, w1p, b1c, w2c, b2c, nslice, nper):
                    # nf: input feature count (partitions), per: nodes/sample
                    for s0, ns in _sample_blocks(BSH, nper):
                        w = ns * per
                        ps1 = ips.tile([32, 512], F32, tag="hid_ps")
                        nc.tensor.matmul(
                            ps1[:, :w],
                            mg[0:nf, w1c:w1c + 32],
                            xtile[0:nf, s0 * per:(s0 + ns) * per],
                            start=True, stop=True)
                        t1 = isb.tile([32, 512], F32, tag="t1")
                        nc.vector.tensor_scalar_add(
                            out=t1[:, :w], in0=ps1[:, :w], scalar1=mg[0:32, b1c:b1c + 1])
                        t2 = isb.tile([32, 512], F32, tag="t2")
                        nc.vector.tensor_scalar_mul(
                            out=t2[:, :w], in0=t1[:, :w], scalar1=0.01)
                        hd = isb.tile([32, 512], F32, tag="hd")
                        nc.vector.tensor_tensor(
                            out=hd[:, :w], in0=t1[:, :w], in1=t2[:, :w], op=AluOpType.max)
                        ps2 = ips.tile([128, 512], F32, tag="emb_ps")
                        nc.tensor.matmul(
                            ps2[:, :w], mg[0:32, w2c:w2c + 128], hd[0:32, :w],
                            start=True, stop=True)
                        dst = hT3[:, s0:s0 + ns, nslice[0]:nslice[1]]
                        src = ps2[:, :w].rearrange("p (s n) -> p s n", n=per)
                        nc.vector.tensor_scalar_add(
                            out=dst, in0=src, scalar1=mg[:, b2c:b2c + 1])

                mlp_phase(xi, 6, IH, W1INT, 6, B1INT, W2INT, B2INT, (0, IH), 6)
                mlp_phase(xl, 8, LH, W1LEAF, 8, B1LEAF, W2LEAF, B2LEAF, (IH, IH + LH), 10)
                mlp_phase(xn, 6, 1, W1NEXT, 6, B1NEXT, W2NEXT, B2NEXT, (IH + LH, NN), 128)

            # ---------------- transformer layers ----------------
            def bn_block(gcol, bcol, bn_idx):
                with tc.tile_pool(name=f"bn{bn_idx}", bufs=1) as bp:
                    stats = bn_stats_tiles[bn_idx]
                    mv = bp.tile([128, 2], F32)
                    nc.vector.bn_aggr(out=mv, in_=stats[:])
                    ccin = bp.tile([128, 2], F32)
                    nc.vector.tensor_scalar_mul(
                        out=ccin[:, 0:1], in0=mv[:, 0:1], scalar1=float(NPOS))
                    msq = bp.tile([128, 1], F32)
                    nc.vector.tensor_tensor(
                        out=msq, in0=mv[:, 0:1], in1=mv[:, 0:1], op=AluOpType.mult)
                    e2 = bp.tile([128, 1], F32)
                    nc.vector.tensor_tensor(
                        out=e2, in0=mv[:, 1:2], in1=msq[:], op=AluOpType.add)
                    nc.vector.tensor_scalar_mul(
                        out=ccin[:, 1:2], in0=e2[:], scalar1=float(NPOS))
                    cin = drampool.tile([128, 2], F32, tag=f"cc_in{bn_idx}")
                    cout = drampool.tile([128, 2], F32, tag=f"cc_out{bn_idx}")
                    nc.gpsimd.dma_start(out=cin[:], in_=ccin[:])
                    if ncores > 1:
                        nc.gpsimd.collective_compute(
                            "AllReduce", AluOpType.add,
                            replica_groups=[list(range(ncores))],
                            ins=[cin.opt()], outs=[cout.opt()])
                    else:
                        nc.gpsimd.dma_start(out=cout[:], in_=cin[:])
                    ccg = bp.tile([128, 2], F32)
                    nc.gpsimd.dma_start(out=ccg[:], in_=cout[:])
                    gl = bp.tile([128, 4], F32)
                    inv_n = 1.0 / float(NTOT)
                    nc.vector.tensor_scalar_mul(out=gl[:, 0:1], in0=ccg[:, 0:1], scalar1=inv_n)
                    nc.vector.tensor_scalar_mul(out=gl[:, 1:2], in0=ccg[:, 1:2], scalar1=inv_n)
                    nc.vector.tensor_tensor(
                        out=gl[:, 2:3], in0=gl[:, 0:1], in1=gl[:, 0:1], op=AluOpType.mult)
                    nc.vector.tensor_tensor(
                        out=gl[:, 3:4], in0=gl[:, 1:2], in1=gl[:, 2:3], op=AluOpType.subtract)
                    nc.vector.tensor_scalar_add(
                        out=gl[:, 3:4], in0=gl[:, 3:4], scalar1=1e-5)
                    nc.scalar.activation(out=gl[:, 3:4], in_=gl[:, 3:4], func=AF.Sqrt)
                    nc.vector.reciprocal(out=gl[:, 3:4], in_=gl[:, 3:4])
                    scsh = bp.tile([128, 2], F32)
                    nc.vector.tensor_scalar_mul(
                        out=scsh[:, 0:1], in0=gl[:, 3:4], scalar1=mg[:, gcol:gcol + 1])
                    t2 = bp.tile([128, 1], F32)
                    nc.vector.tensor_tensor(
                        out=t2, in0=gl[:, 0:1], in1=scsh[:, 0:1], op=AluOpType.mult)
                    nc.vector.tensor_scalar(
                        out=scsh[:, 1:2], in0=t2[:], scalar1=-1.0,
                        scalar2=mg[:, bcol:bcol + 1], op0=AluOpType.mult, op1=AluOpType.add)
                    for c0, w in _col_blocks(NPOS, 1048):
                        nc.gpsimd.tensor_scalar(
                            out=hT[:, c0:c0 + w], in0=hT[:, c0:c0 + w],
                            scalar1=scsh[:, 0:1], scalar2=scsh[:, 1:2],
                            op0=AluOpType.mult, op1=AluOpType.add)

            for l in range(2):
                with tc.tile_pool(name=f"attsb{l}", bufs=1) as ap, \
                     tc.tile_pool(name=f"attsb2_{l}", bufs=2) as ap2, \
                     tc.tile_pool(name=f"attms{l}", bufs=6) as apm:
                    att = ap.tile([128, NPOS], F32, tag="attnout")
                    ctx_pools = [
                        tc.tile_pool(name=f"attps{l}", bufs=1, space="PSUM"),
                        tc.tile_pool(name=f"attps2_{l}", bufs=2, space="PSUM"),
                        tc.tile_pool(name=f"attps3_{l}", bufs=2, space="PSUM"),
                        tc.tile_pool(name=f"attps4_{l}", bufs=1, space="PSUM"),
                    ]
                    pp1 = ctx_pools[0].__enter__()
                    pp2 = ctx_pools[1].__enter__()
                    pp3 = ctx_pools[2].__enter__()
                    pp4 = ctx_pools[3].__enter__()
                    em1 = [0]
                    wo_done = [0]
                    wo_blocks = _col_blocks(NPOS, 512)

                    def emit_wo(s_done):
                        # wo is folded into wv host-side (v' = h @ (wv@wo)),
                        # so att already holds mh^T: residual is a plain add
                        while wo_done[0] < len(wo_blocks):
                            c0, w = wo_blocks[wo_done[0]]
                            if c0 + w > 131 * s_done:
                                break
                            nc.vector.tensor_tensor(
                                out=hT[:, c0:c0 + w], in0=hT[:, c0:c0 + w],
                                in1=att[:, c0:c0 + w], op=AluOpType.add)
                            emit_stats(2 * l, em1, c0 + w)
                            wo_done[0] += 1
                    for s0, ns in _sample_blocks(BSH, 3):
                        cb, w = s0 * NN, ns * NN
                        qps = pp1.tile([128, 393], F32, tag="qps")
                        nc.tensor.matmul(
                            qps[:, :w], mg[:, WQ[l]:WQ[l] + 128], hT[:, cb:cb + w],
                            start=True, stop=True)
                        kps = pp1.tile([128, 393], F32, tag="kps")
                        nc.tensor.matmul(
                            kps[:, :w], mg[:, WK[l]:WK[l] + 128], hT[:, cb:cb + w],
                            start=True, stop=True)
                        qT = ap2.tile([128, 393], F32, tag="qT")
                        nc.scalar.copy(out=qT[:, :w], in_=qps[:, :w])
                        kT = ap2.tile([128, 393], F32, tag="kT")
                        nc.scalar.copy(out=kT[:, :w], in_=kps[:, :w])

                        sc = ap2.tile([128, 393], F32, tag="sc")
                        scs = ap2.tile([3, 393], F32, tag="scs")
                        for i in range(ns):
                            s, c = s0 + i, i * NN
                            sp = pp2.tile([128, 262], F32, tag="scps")
                            nc.tensor.matmul(
                                sp[:, 0:131], qT[:, c:c + 128], kT[:, c:c + 131],
                                start=True, stop=False)
                            msk = apm.tile([1, NN], BF16, tag="msk")
                            nc.sync.dma_start(out=msk, in_=masksbf[s:s + 1, :])
                            nc.tensor.matmul(
                                sp[:, 0:131], ob[0:1, 0:128], msk[0:1, :],
                                start=False, stop=True)
                            nc.tensor.matmul(
                                sp[0:3, 131:262], qT[:, c + 128:c + 131], kT[:, c:c + 131],
                                start=True, stop=False)
                            nc.tensor.matmul(
                                sp[0:3, 131:262], ob[0:1, 0:3], msk[0:1, :],
                                start=False, stop=True)
                            nc.vector.tensor_copy(out=sc[:, c:c + 131], in_=sp[:, 0:131])
                            nc.vector.tensor_copy(out=scs[:, c:c + 131], in_=sp[0:3, 131:262])
                        # batched softmax (no max-subtraction; scores bounded)
                        ex = ap2.tile([128, 393], F32, tag="ex")
                        nc.scalar.activation(out=ex[:, :w], in_=sc[:, :w], func=AF.Exp)
                        exs = ap2.tile([3, 393], F32, tag="exs")
                        nc.scalar.activation(out=exs[:, :w], in_=scs[:, :w], func=AF.Exp)
                        sm = ap2.tile([128, 3], F32, tag="sm")
                        nc.vector.reduce_sum(
                            out=sm[:, :ns],
                            in_=ex[:, :w].rearrange("p (s n) -> p s n", n=NN), axis=AX.X)
                        sms = ap2.tile([3, 3], F32, tag="sms")
                        nc.vector.reduce_sum(
                            out=sms[:, :ns],
                            in_=exs[:, :w].rearrange("p (s n) -> p s n", n=NN), axis=AX.X)
                        nc.vector.reciprocal(out=sm[:, :ns], in_=sm[:, :ns])
                        nc.vector.reciprocal(out=sms[:, :ns], in_=sms[:, :ns])
                        for i in range(ns):
                            s, c = s0 + i, i * NN
                            sb = s * NN
                            nc.vector.tensor_scalar_mul(
                                out=ex[:, c:c + 131], in0=ex[:, c:c + 131],
                                scalar1=sm[:, i:i + 1])
                            nc.vector.tensor_scalar_mul(
                                out=exs[:, c:c + 131], in0=exs[:, c:c + 131],
                                scalar1=sms[:, i:i + 1])
                            ta = pp3.tile([128, 262], F32, tag="taps")
                            nc.tensor.transpose(ta[:, 0:128], ex[:, c:c + 128], ident)
                            nc.tensor.transpose(
                                ta[:, 128:131], exs[0:3, c:c + 128], ident[0:3, 0:3])
                            tsav = pp4.tile([128, 393], F32, tag="tsav")
                            ts = tsav
                            nc.tensor.transpose(ts[0:3, 0:128], ex[:, c + 128:c + 131], ident)
                            nc.tensor.transpose(
                                ts[0:3, 128:131], exs[0:3, c + 128:c + 131], ident[0:3, 0:3])
                            # v directly from hT (rows = keys)
                            nc.tensor.matmul(
                                ta[:, 131:259], hT[:, sb:sb + 128], mg[:, WV[l]:WV[l] + 128],
                                start=True, stop=True)
                            nc.tensor.matmul(
                                ts[0:3, 131:259], hT[:, sb + 128:sb + 131],
                                mg[:, WV[l]:WV[l] + 128], start=True, stop=True)
                            aT = ap2.tile([128, 262], F32, tag="aT")
                            nc.vector.tensor_copy(out=aT, in_=ta)
                            aS = ap2.tile([3, 262], F32, tag="aS")
                            nc.vector.tensor_copy(out=aS, in_=tsav[0:3, 0:262])
                            av = tsav[:, 262:393]
                            nc.tensor.matmul(
                                av, aT[:, 131:259], aT[:, 0:131], start=True, stop=False)
                            nc.tensor.matmul(
                                av, aS[0:3, 131:259], aS[0:3, 0:131], start=False, stop=True)
                            nc.scalar.copy(out=att[:, sb:sb + NN], in_=av)
                        emit_wo(s0 + ns)
                    emit_wo(BSH + 4)
                    assert wo_done[0] == len(wo_blocks) and em1[0] == 64
                    for cp_ in reversed(ctx_pools):
                        cp_.__exit__(None, None, None)
                bn_block(N1G[l], N1B[l], 2 * l)
                # FF block (ffb2 dropped: per-channel constant is absorbed by BN2)
                with tc.tile_pool(name=f"ffsb{l}", bufs=2) as fsb, \
                     tc.tile_pool(name=f"ffps{l}", bufs=2, space="PSUM") as fps:
                    em2 = [0]
                    for c0, w in _col_blocks(NPOS, 512):
                        f2ps = fps.tile([128, 512], F32, tag="f2ps")
                        for c in range(4):
                            f1ps = fps.tile([128, 512], F32, tag="f1ps")
                            nc.tensor.matmul(
                                f1ps[:, :w], mg[:, FW1[l] + c * 128:FW1[l] + c * 128 + 128],
                                hT[:, c0:c0 + w], start=True, stop=True)
                            rl = fsb.tile([128, 512], F32, tag="rl")
                            nc.scalar.activation(
                                out=rl[:, :w], in_=f1ps[:, :w], func=AF.Relu,
                                bias=mg[:, FB1[l] + c:FB1[l] + c + 1])
                            nc.tensor.matmul(
                                f2ps[:, :w], mg[:, FW2P[l] + c * 128:FW2P[l] + c * 128 + 128],
                                rl[:, :w], start=(c == 0), stop=(c == 3))
                        nc.vector.tensor_tensor(
                            out=hT[:, c0:c0 + w], in0=hT[:, c0:c0 + w],
                            in1=f2ps[:, :w], op=AluOpType.add)
                        emit_stats(2 * l + 1, em2, c0 + w)
                bn_block(N2G[l], N2B[l], 2 * l + 1)

            # ---------------- final head ----------------
            with tc.tile_pool(name="finsb", bufs=1) as fn, \
                 tc.tile_pool(name="finsb2", bufs=2) as fn2, \
                 tc.tile_pool(name="finps", bufs=2, space="PSUM") as fnp, \
                 tc.tile_pool(name="finps1", bufs=1, space="PSUM") as fnp1:
                vb = fn.tile([1, NPOS], BF16, tag="vb")
                nc.sync.dma_start(out=vb, in_=validbf[:])
                # emb = h * valid (zero invalid nodes), in place
                for c0, w in _col_blocks(NPOS, 512):
                    bps = fnp.tile([128, 512], F32, tag="bps")
                    nc.tensor.matmul(
                        bps[:, :w], ob[0:1, 0:128], vb[0:1, c0:c0 + w],
                        start=True, stop=True)
                    nc.vector.tensor_tensor(
                        out=hT[:, c0:c0 + w], in0=hT[:, c0:c0 + w],
                        in1=bps[:, :w], op=AluOpType.mult)
                gr = fn.tile([128, BSH], F32, tag="gr")
                nc.vector.reduce_sum(out=gr, in_=hT3, axis=AX.X)
                cps = fnp1.tile([128, 128], F32, tag="cps")
                nc.tensor.matmul(
                    cps[:], mg[:, WCTX:WCTX + 128], gr[:], start=True, stop=True)
                ctxT = fn.tile([128, BSH], F32, tag="ctxT")
                nc.vector.tensor_copy(out=ctxT, in_=cps)
                # comp = emb . (Wn @ ctx): one [D,BSH] matmul replaces the
                # whole gK projection (mega holds Wn^T so lhsT.T = Wn)
                ups = fnp1.tile([128, BSH], F32, tag="ups")
                nc.tensor.matmul(
                    ups[:], mg[:, WNODE:WNODE + 128], ctxT[:], start=True, stop=True)
                uT = fn.tile([128, BSH], F32, tag="uT")
                nc.vector.tensor_copy(out=uT, in_=ups)
                for s0, ns in _sample_blocks(BSH, 3):
                    cp = fnp.tile([1, 393], F32, tag="cp")
                    for i in range(ns):
                        s = s0 + i
                        nc.tensor.matmul(
                            cp[0:1, i * NN:i * NN + NN], uT[:, s:s + 1],
                            hT[:, s * NN:s * NN + NN], start=True, stop=True)
                    stg = fn2.tile([1, 393], F32, tag="stg")
                    nc.vector.tensor_copy(out=stg[0:1, :ns * NN], in_=cp[0:1, :ns * NN])
                    src3 = stg[0:1, :ns * NN].rearrange("p (s n) -> p s n", n=NN)
                    nc.sync.dma_start(
                        out=OutComp[0:1, s0 * LH:(s0 + ns) * LH],
                        in_=src3[:, :, IH:IH + LH])
                tps = fnp1.tile([128, 128], F32, tag="tps")
                nc.tensor.transpose(tps[:], ctxT[:], ident)
                co = fn.tile([BSH, D], F32, tag="co")
                nc.vector.tensor_scalar_mul(
                    out=co, in0=tps[:], scalar1=mg[:, INVLEN:INVLEN + 1])
                nc.sync.dma_start(out=OutCtx[:], in_=co[:])
    return nc


def _prep_inputs(input, params):
    """Host-side preprocessing: shard + transpose + pack mega weight tile."""
    inp = np.ascontiguousarray(np.asarray(input, dtype=np.float32))
    p = params

    def f32(x):
        return np.asarray(x, dtype=np.float32)

    mega_common = np.zeros((128, MEGA_COLS), dtype=np.float32)
    mega_common[:, IDC:IDC + 128] = np.eye(128, dtype=np.float32)
    for l in range(2):
        lyr = p["layers"][l]
        mega_common[:, WQ[l]:WQ[l] + 128] = f32(lyr["wq"]) * np.float32(ISD)
        mega_common[:, WK[l]:WK[l] + 128] = f32(lyr["wk"])
        mega_common[:, WV[l]:WV[l] + 128] = f32(lyr["wv"]) @ f32(lyr["wo"])
        mega_common[:, WO[l]:WO[l] + 128] = f32(lyr["wo"])
        mega_common[:, FW1[l]:FW1[l] + 512] = f32(lyr["ff_w1"])
        for c in range(4):
            mega_common[:, FW2P[l] + c * 128:FW2P[l] + (c + 1) * 128] = \
                f32(lyr["ff_w2"])[c * 128:(c + 1) * 128, :]
            mega_common[:, FB1[l] + c] = f32(lyr["ff_b1"])[c * 128:(c + 1) * 128]
        mega_common[:, N1G[l]] = f32(lyr["n1_g"])
        mega_common[:, N1B[l]] = f32(lyr["n1_b"])
        mega_common[:, N2G[l]] = f32(lyr["n2_g"])
        mega_common[:, N2B[l]] = f32(lyr["n2_b"])
    mega_common[:, WCTX:WCTX + 128] = f32(p["w_ctx"])
    mega_common[:, WNODE:WNODE + 128] = \
        np.ascontiguousarray((f32(p["w_node"])[:, :D] * np.float32(ISD)).T)
    mega_common[0:32, W2INT:W2INT + 128] = f32(p["int_w2"])
    mega_common[0:32, W2LEAF:W2LEAF + 128] = f32(p["leaf_w2"])
    mega_common[0:32, W2NEXT:W2NEXT + 128] = f32(p["next_w2"])
    mega_common[0:6, W1INT:W1INT + 32] = f32(p["int_w1"])
    mega_common[0:8, W1LEAF:W1LEAF + 32] = f32(p["leaf_w1"])
    mega_common[0:6, W1NEXT:W1NEXT + 32] = f32(p["next_w1"])
    mega_common[0:32, B1INT] = f32(p["int_b1"])
    mega_common[0:32, B1LEAF] = f32(p["leaf_b1"])
    mega_common[0:32, B1NEXT] = f32(p["next_b1"])
    mega_common[:, B2INT] = f32(p["int_b2"])
    mega_common[:, B2LEAF] = f32(p["leaf_b2"])
    mega_common[:, B2NEXT] = f32(p["next_b2"])

    valid = inp[..., 8]
    in_maps = []
    for c in range(NCORES):
        sl = slice(c * BSH, (c + 1) * BSH)
        xs = inp[sl]
        vs = valid[sl]
        mega = mega_common.copy()
        mega[:, INVLEN] = (1.0 / vs.sum(axis=1)).astype(np.float32)
        mega[:, LEAFM:LEAFM + LH] = vs[:, IH:IH + LH]
        xi = np.ascontiguousarray(
            xs[:, :IH, :6].reshape(BSH * IH, 6).T)
        xl = np.ascontiguousarray(
            xs[:, IH:IH + LH, :8].reshape(BSH * LH, 8).T)
        xn = np.ascontiguousarray(
            xs[:, IH + LH:, :6].reshape(BSH, 6).T)
        mask_add = np.where(vs < 0.5, np.float32(-1e9), np.float32(0.0))
        in_maps.append({
            "mega": mega,
            "xint": xi, "xleaf": xl, "xnext": xn,
            "masksbf": mask_add.astype(ml_dtypes.bfloat16),
            "validbf": (vs >= 0.5).astype(ml_dtypes.bfloat16).reshape(1, NPOS),
            "onesbf": np.ones((1, 128), dtype=ml_dtypes.bfloat16),
        })
    return in_maps


_NC_CACHE = None


def run_device(input, params, trace=False):
    global _NC_CACHE
    if _NC_CACHE is None:
        _NC_CACHE = build_bass()
        _NC_CACHE.finalize()
    nc = _NC_CACHE
    in_maps = _prep_inputs(input, params)
    res = run_bass_kernel_spmd(nc, in_maps, core_ids=list(range(NCORES)), trace=trace)
    comp = np.concatenate(
        [r["OutComp"].reshape(BSH, LH) for r in res.results], axis=0)
    ctx = np.concatenate([r["OutCtx"] for r in res.results], axis=0)
    return comp, ctx, res


def host_finish(comp, input):
    """comp: raw ctx.gK/sqrt(D) dot products for leaf nodes, [B, 50]."""
    valid = np.asarray(input, np.float32)[..., 8]
    inv_len = (1.0 / valid.sum(axis=1, dtype=np.float32)).astype(np.float32)
    logits = (np.tanh(comp * inv_len[:, None]) * np.float32(10.0)).astype(np.float32)
    p = np.exp(logits - logits.max(axis=1, keepdims=True)).astype(np.float32)
    p = (p / p.sum(axis=1, keepdims=True)).astype(np.float32)
    masked = p * valid[:, IH:IH + LH]
    probs = (masked / masked.sum(axis=1, keepdims=True)).astype(np.float32)
    return probs


def kernel(input, params):
    comp, ctx, _ = run_device(input, params)
    probs = host_finish(comp, input)
    # host finishing ops (cheap [B,50] reductions, fp32 to match reference)
    logp = np.log(np.clip(probs, np.float32(1e-38), None)).astype(np.float32)
    logp_safe = np.where(probs > 0, logp, np.float32(0.0))
    entropy = -(probs * logp_safe).sum(axis=-1)
    selected = np.argmax(probs, axis=-1).astype(np.int32)
    alp = np.take_along_axis(logp, selected[:, None].astype(np.int64), axis=1)
    return (alp.astype(np.float32), selected, entropy.astype(np.float32),
            ctx.astype(np.float32), probs.astype(np.float32))


# revision 27
# speedup vs baseline: 1.0478x; 1.0478x over previous
"""Trainium2 Bass kernel for nn_AttentionModel (sparse_attention).

Strategy: pure data parallel over 8 NeuronCores (128 samples each), weights
replicated. BatchNorm statistics are made *exact* (matching single-device
reference) via a tiny AllReduce of per-core [sum, sumsq] per channel.

Device layout: activations live transposed as hT [D=128 partitions,
positions = sample*131+node]. Per-sample attention (131x131, single head,
dim 128) is computed with PE matmuls; key masking is applied with a rank-1
accumulating matmul (ones^T @ mask_row) so no partition-broadcast is needed.
Softmax skips max-subtraction (scores are bounded: |s| < 13 measured, and
masked entries are -1e9 -> exp underflows to exactly 0).

Final probs + ctx are computed on device; alp/selected/entropy are cheap
[B,50] reductions done on host in fp32.

fp32 everywhere: top-2 prob gaps go down to 2.7e-4, so bf16/fp32r matmuls
would flip argmax outputs.
"""

import sys

sys.path.insert(0, "/opt/trn_rl_repo")

import numpy as np
import ml_dtypes

import concourse.bass as bass
import concourse.tile as tile
from concourse import bacc
from concourse import mybir
from concourse.alu_op_type import AluOpType
from concourse.bass_utils import run_bass_kernel_spmd

F32 = mybir.dt.float32
BF16 = mybir.dt.bfloat16
AF = mybir.ActivationFunctionType
AX = mybir.AxisListType

NCORES = 8
B = 1024
BSH = B // NCORES  # 128 samples per core
IH, LH, NN = 80, 50, 131
D = 128
FFH = 512
NPOS = BSH * NN  # 16768
NTOT = B * NN  # 134144 positions globally
ISD = 1.0 / np.sqrt(np.float32(D))

# ---- mega weight-tile column layout (fp32 [128, MEGA_COLS]) ----
_c = 0
def _alloc(n):
    global _c
    off = _c
    _c += n
    return off

IDC = _alloc(128)
WQ = [_alloc(128) for _ in range(2)]
WK = [_alloc(128) for _ in range(2)]
WV = [_alloc(128) for _ in range(2)]
WO = [_alloc(128) for _ in range(2)]
FW1 = [_alloc(512) for _ in range(2)]
FW2P = [_alloc(512) for _ in range(2)]  # packed: chunk c at +c*128 = ffw2[c*128:(c+1)*128, :]
WCTX = _alloc(128)
WNODE = _alloc(128)  # first D cols of w_node, scaled by 1/sqrt(D)
W2INT = _alloc(128)  # rows 0:32
W2LEAF = _alloc(128)
W2NEXT = _alloc(128)
W1INT = _alloc(32)  # rows 0:6
W1LEAF = _alloc(32)  # rows 0:8
W1NEXT = _alloc(32)  # rows 0:6
B1INT = _alloc(1)  # rows 0:32
B1LEAF = _alloc(1)
B1NEXT = _alloc(1)
B2INT = _alloc(1)  # rows 0:128
B2LEAF = _alloc(1)
B2NEXT = _alloc(1)
FB1 = [_alloc(4) for _ in range(2)]  # chunk c at +c, rows = ffb1[c*128:(c+1)*128]
N1G = [_alloc(1) for _ in range(2)]
N1B = [_alloc(1) for _ in range(2)]
N2G = [_alloc(1) for _ in range(2)]
N2B = [_alloc(1) for _ in range(2)]
INVLEN = _alloc(1)  # per-core 1/valid_length, [128,1]
LEAFM = _alloc(LH)  # per-core leaf valid mask (0/1) [128,50]
MEGA_COLS = ((_c + 31) // 32) * 32


def _sample_blocks(bs, nper):
    out = []
    s0 = 0
    while s0 < bs:
        out.append((s0, min(nper, bs - s0)))
        s0 += nper
    return out


def _col_blocks(total, w):
    out = []
    c0 = 0
    while c0 < total:
        out.append((c0, min(w, total - c0)))
        c0 += w
    return out


def build_bass(ncores=NCORES):
    nc = bacc.Bacc("TRN2", num_devices=ncores, target_bir_lowering=False)

    mega = nc.dram_tensor("mega", [128, MEGA_COLS], F32, kind="ExternalInput")
    xint = nc.dram_tensor("xint", [6, BSH * IH], F32, kind="ExternalInput")
    xleaf = nc.dram_tensor("xleaf", [8, BSH * LH], F32, kind="ExternalInput")
    xnext = nc.dram_tensor("xnext", [6, BSH], F32, kind="ExternalInput")
    masksbf = nc.dram_tensor("masksbf", [BSH, NN], BF16, kind="ExternalInput")
    validbf = nc.dram_tensor("validbf", [1, NPOS], BF16, kind="ExternalInput")
    onesbf = nc.dram_tensor("onesbf", [1, 128], BF16, kind="ExternalInput")
    OutComp = nc.dram_tensor("OutComp", [1, BSH * LH], F32, kind="ExternalOutput")
    OutCtx = nc.dram_tensor("OutCtx", [BSH, D], F32, kind="ExternalOutput")

    with tile.TileContext(nc) as tc:
        with tc.tile_pool(name="persist", bufs=1) as persist, \
             tc.tile_pool(name="drampool", bufs=1, space="DRAM") as drampool:
            mg = persist.tile([128, MEGA_COLS], F32)
            nc.sync.dma_start(out=mg, in_=mega[:])
            ob = persist.tile([1, 128], BF16)
            nc.sync.dma_start(out=ob, in_=onesbf[:])
            ident = mg[:, IDC:IDC + 128]

            # PE fences: absorb the mega/ones DMA waits once so later matmuls
            # carry at most one semaphore wait (fp32 matmul HW limit).
            with tc.tile_pool(name="fencep", bufs=1, space="PSUM") as fencep:
                f1 = fencep.tile([1, 128], F32)
                nc.tensor.transpose(f1[:], mg[:, IDC:IDC + 1], ident)
                f2 = fencep.tile([1, 1], F32)
                nc.tensor.matmul(f2[:], ob[0:1, 0:1], ob[0:1, 0:1], start=True, stop=True)

            hT = persist.tile([128, NPOS], F32)
            hT3 = hT[:].rearrange("p (s n) -> p s n", n=NN)
            hTv = hT[:].rearrange("p (g c) -> p g c", c=262)
            bn_stats_tiles = []
            for i in range(4):
                bnst = persist.tile([128, 64, 6], F32, tag=f"bnst{i}")
                bn_stats_tiles.append(bnst)

            def emit_stats(bn_idx, emitted, upto_col):
                # emit bn_stats for every 262-col group fully covered by
                # residuals completed up to upto_col (pipelines the BN
                # reduction with the preceding residual loop)
                stats = bn_stats_tiles[bn_idx]
                while emitted[0] < 64 and 262 * (emitted[0] + 1) <= upto_col:
                    g = emitted[0]
                    nc.vector.bn_stats(out=stats[:, g, :], in_=hTv[:, g, :])
                    emitted[0] += 1

            # ---------------- init embeddings ----------------
            with tc.tile_pool(name="initin", bufs=1) as iin, \
                 tc.tile_pool(name="initsb", bufs=2) as isb, \
                 tc.tile_pool(name="initps", bufs=2, space="PSUM") as ips:
                xi = iin.tile([6, BSH * IH], F32, tag="xin_i")
                nc.sync.dma_start(out=xi, in_=xint[:])
                xl = iin.tile([8, BSH * LH], F32, tag="xin_l")
                nc.sync.dma_start(out=xl, in_=xleaf[:])
                xn = iin.tile([6, BSH], F32, tag="xin_n")
                nc.sync.dma_start(out=xn, in_=xnext[:])

                def mlp_phase(xtile, nf, per, w1c, w1p, b1c, w2c, b2c, nslice, nper):
                    # nf: input feature count (partitions), per: nodes/sample
                    for s0, ns in _sample_blocks(BSH, nper):
                        w = ns * per
                        ps1 = ips.tile([32, 512], F32, tag="hid_ps")
                        nc.tensor.matmul(
                            ps1[:, :w],
                            mg[0:nf, w1c:w1c + 32],
                            xtile[0:nf, s0 * per:(s0 + ns) * per],
                            start=True, stop=True)
                        t1 = isb.tile([32, 512], F32, tag="t1")
                        nc.vector.tensor_scalar_add(
                            out=t1[:, :w], in0=ps1[:, :w], scalar1=mg[0:32, b1c:b1c + 1])
                        t2 = isb.tile([32, 512], F32, tag="t2")
                        nc.vector.tensor_scalar_mul(
                            out=t2[:, :w], in0=t1[:, :w], scalar1=0.01)
                        hd = isb.tile([32, 512], F32, tag="hd")
                        nc.vector.tensor_tensor(
                            out=hd[:, :w], in0=t1[:, :w], in1=t2[:, :w], op=AluOpType.max)
                        ps2 = ips.tile([128, 512], F32, tag="emb_ps")
                        nc.tensor.matmul(
                            ps2[:, :w], mg[0:32, w2c:w2c + 128], hd[0:32, :w],
                            start=True, stop=True)
                        dst = hT3[:, s0:s0 + ns, nslice[0]:nslice[1]]
                        src = ps2[:, :w].rearrange("p (s n) -> p s n", n=per)
                        nc.vector.tensor_scalar_add(
                            out=dst, in0=src, scalar1=mg[:, b2c:b2c + 1])

                mlp_phase(xi, 6, IH, W1INT, 6, B1INT, W2INT, B2INT, (0, IH), 6)
                mlp_phase(xl, 8, LH, W1LEAF, 8, B1LEAF, W2LEAF, B2LEAF, (IH, IH + LH), 10)
                mlp_phase(xn, 6, 1, W1NEXT, 6, B1NEXT, W2NEXT, B2NEXT, (IH + LH, NN), 128)

            # ---------------- transformer layers ----------------
            def bn_block(gcol, bcol, bn_idx):
                with tc.tile_pool(name=f"bn{bn_idx}", bufs=1) as bp:
                    stats = bn_stats_tiles[bn_idx]
                    mv = bp.tile([128, 2], F32)
                    nc.vector.bn_aggr(out=mv, in_=stats[:])
                    ccin = bp.tile([128, 2], F32)
                    nc.vector.tensor_scalar_mul(
                        out=ccin[:, 0:1], in0=mv[:, 0:1], scalar1=float(NPOS))
                    msq = bp.tile([128, 1], F32)
                    nc.vector.tensor_tensor(
                        out=msq, in0=mv[:, 0:1], in1=mv[:, 0:1], op=AluOpType.mult)
                    e2 = bp.tile([128, 1], F32)
                    nc.vector.tensor_tensor(
                        out=e2, in0=mv[:, 1:2], in1=msq[:], op=AluOpType.add)
                    nc.vector.tensor_scalar_mul(
                        out=ccin[:, 1:2], in0=e2[:], scalar1=float(NPOS))
                    cin = drampool.tile([128, 2], F32, tag=f"cc_in{bn_idx}")
                    cout = drampool.tile([128, 2], F32, tag=f"cc_out{bn_idx}")
                    nc.gpsimd.dma_start(out=cin[:], in_=ccin[:])
                    if ncores > 1:
                        nc.gpsimd.collective_compute(
                            "AllReduce", AluOpType.add,
                            replica_groups=[list(range(ncores))],
                            ins=[cin.opt()], outs=[cout.opt()])
                    else:
                        nc.gpsimd.dma_start(out=cout[:], in_=cin[:])
                    ccg = bp.tile([128, 2], F32)
                    nc.gpsimd.dma_start(out=ccg[:], in_=cout[:])
                    gl = bp.tile([128, 4], F32)
                    inv_n = 1.0 / float(NTOT)
                    nc.vector.tensor_scalar_mul(out=gl[:, 0:1], in0=ccg[:, 0:1], scalar1=inv_n)
                    nc.vector.tensor_scalar_mul(out=gl[:, 1:2], in0=ccg[:, 1:2], scalar1=inv_n)
                    nc.vector.tensor_tensor(
                        out=gl[:, 2:3], in0=gl[:, 0:1], in1=gl[:, 0:1], op=AluOpType.mult)
                    nc.vector.tensor_tensor(
                        out=gl[:, 3:4], in0=gl[:, 1:2], in1=gl[:, 2:3], op=AluOpType.subtract)
                    nc.vector.tensor_scalar_add(
                        out=gl[:, 3:4], in0=gl[:, 3:4], scalar1=1e-5)
                    nc.scalar.activation(out=gl[:, 3:4], in_=gl[:, 3:4], func=AF.Sqrt)
                    nc.vector.reciprocal(out=gl[:, 3:4], in_=gl[:, 3:4])
                    scsh = bp.tile([128, 2], F32)
                    nc.vector.tensor_scalar_mul(
                        out=scsh[:, 0:1], in0=gl[:, 3:4], scalar1=mg[:, gcol:gcol + 1])
                    t2 = bp.tile([128, 1], F32)
                    nc.vector.tensor_tensor(
                        out=t2, in0=gl[:, 0:1], in1=scsh[:, 0:1], op=AluOpType.mult)
                    nc.vector.tensor_scalar(
                        out=scsh[:, 1:2], in0=t2[:], scalar1=-1.0,
                        scalar2=mg[:, bcol:bcol + 1], op0=AluOpType.mult, op1=AluOpType.add)
                    for c0, w in _col_blocks(NPOS, 1048):
                        nc.gpsimd.tensor_scalar(
                            out=hT[:, c0:c0 + w], in0=hT[:, c0:c0 + w],
                            scalar1=scsh[:, 0:1], scalar2=scsh[:, 1:2],
                            op0=AluOpType.mult, op1=AluOpType.add)

            for l in range(2):
                with tc.tile_pool(name=f"attsb{l}", bufs=1) as ap, \
                     tc.tile_pool(name=f"attsb2_{l}", bufs=2) as ap2, \
                     tc.tile_pool(name=f"attms{l}", bufs=6) as apm:
                    att = ap.tile([128, NPOS], F32, tag="attnout")
                    ctx_pools = [
                        tc.tile_pool(name=f"attps{l}", bufs=1, space="PSUM"),
                        tc.tile_pool(name=f"attps2_{l}", bufs=2, space="PSUM"),
                        tc.tile_pool(name=f"attps3_{l}", bufs=2, space="PSUM"),
                        tc.tile_pool(name=f"attps4_{l}", bufs=1, space="PSUM"),
                    ]
                    pp1 = ctx_pools[0].__enter__()
                    pp2 = ctx_pools[1].__enter__()
                    pp3 = ctx_pools[2].__enter__()
                    pp4 = ctx_pools[3].__enter__()
                    em1 = [0]
                    wo_done = [0]
                    wo_blocks = _col_blocks(NPOS, 512)

                    def emit_wo(s_done):
                        # wo is folded into wv host-side (v' = h @ (wv@wo)),
                        # so att already holds mh^T: residual is a plain add
                        while wo_done[0] < len(wo_blocks):
                            c0, w = wo_blocks[wo_done[0]]
                            if c0 + w > 131 * s_done:
                                break
                            nc.vector.tensor_tensor(
                                out=hT[:, c0:c0 + w], in0=hT[:, c0:c0 + w],
                                in1=att[:, c0:c0 + w], op=AluOpType.add)
                            emit_stats(2 * l, em1, c0 + w)
                            wo_done[0] += 1
                    for s0, ns in _sample_blocks(BSH, 3):
                        cb, w = s0 * NN, ns * NN
                        qps = pp1.tile([128, 393], F32, tag="qps")
                        nc.tensor.matmul(
                            qps[:, :w], mg[:, WQ[l]:WQ[l] + 128], hT[:, cb:cb + w],
                            start=True, stop=True)
                        kps = pp1.tile([128, 393], F32, tag="kps")
                        nc.tensor.matmul(
                            kps[:, :w], mg[:, WK[l]:WK[l] + 128], hT[:, cb:cb + w],
                            start=True, stop=True)
                        qT = ap2.tile([128, 393], F32, tag="qT")
                        nc.scalar.copy(out=qT[:, :w], in_=qps[:, :w])
                        kT = ap2.tile([128, 393], F32, tag="kT")
                        nc.scalar.copy(out=kT[:, :w], in_=kps[:, :w])

                        sc = ap2.tile([128, 393], F32, tag="sc")
                        scs = ap2.tile([3, 393], F32, tag="scs")
                        for i in range(ns):
                            s, c = s0 + i, i * NN
                            sp = pp2.tile([128, 262], F32, tag="scps")
                            nc.tensor.matmul(
                                sp[:, 0:131], qT[:, c:c + 128], kT[:, c:c + 131],
                                start=True, stop=False)
                            msk = apm.tile([1, NN], BF16, tag="msk")
                            nc.sync.dma_start(out=msk, in_=masksbf[s:s + 1, :])
                            nc.tensor.matmul(
                                sp[:, 0:131], ob[0:1, 0:128], msk[0:1, :],
                                start=False, stop=True)
                            nc.tensor.matmul(
                                sp[0:3, 131:262], qT[:, c + 128:c + 131], kT[:, c:c + 131],
                                start=True, stop=False)
                            nc.tensor.matmul(
                                sp[0:3, 131:262], ob[0:1, 0:3], msk[0:1, :],
                                start=False, stop=True)
                            nc.vector.tensor_copy(out=sc[:, c:c + 131], in_=sp[:, 0:131])
                            nc.vector.tensor_copy(out=scs[:, c:c + 131], in_=sp[0:3, 131:262])
                        # batched softmax (no max-subtraction; scores bounded)
                        ex = ap2.tile([128, 393], F32, tag="ex")
                        nc.scalar.activation(out=ex[:, :w], in_=sc[:, :w], func=AF.Exp)
                        exs = ap2.tile([3, 393], F32, tag="exs")
                        nc.scalar.activation(out=exs[:, :w], in_=scs[:, :w], func=AF.Exp)
                        sm = ap2.tile([128, 3], F32, tag="sm")
                        nc.vector.reduce_sum(
                            out=sm[:, :ns],
                            in_=ex[:, :w].rearrange("p (s n) -> p s n", n=NN), axis=AX.X)
                        sms = ap2.tile([3, 3], F32, tag="sms")
                        nc.vector.reduce_sum(
                            out=sms[:, :ns],
                            in_=exs[:, :w].rearrange("p (s n) -> p s n", n=NN), axis=AX.X)
                        nc.vector.reciprocal(out=sm[:, :ns], in_=sm[:, :ns])
                        nc.vector.reciprocal(out=sms[:, :ns], in_=sms[:, :ns])
                        for i in range(ns):
                            s, c = s0 + i, i * NN
                            sb = s * NN
                            nc.vector.tensor_scalar_mul(
                                out=ex[:, c:c + 131], in0=ex[:, c:c + 131],
                                scalar1=sm[:, i:i + 1])
                            nc.vector.tensor_scalar_mul(
                                out=exs[:, c:c + 131], in0=exs[:, c:c + 131],
                                scalar1=sms[:, i:i + 1])
                            ta = pp3.tile([128, 262], F32, tag="taps")
                            nc.tensor.transpose(ta[:, 0:128], ex[:, c:c + 128], ident)
                            nc.tensor.transpose(
                                ta[:, 128:131], exs[0:3, c:c + 128], ident[0:3, 0:3])
                            tsav = pp4.tile([128, 393], F32, tag="tsav")
                            ts = tsav
                            nc.tensor.transpose(ts[0:3, 0:128], ex[:, c + 128:c + 131], ident)
                            nc.tensor.transpose(
                                ts[0:3, 128:131], exs[0:3, c + 128:c + 131], ident[0:3, 0:3])
                            # v directly from hT (rows = keys)
                            nc.tensor.matmul(
                                ta[:, 131:259], hT[:, sb:sb + 128], mg[:, WV[l]:WV[l] + 128],
                                start=True, stop=True)
                            nc.tensor.matmul(
                                ts[0:3, 131:259], hT[:, sb + 128:sb + 131],
                                mg[:, WV[l]:WV[l] + 128], start=True, stop=True)
                            aT = ap2.tile([128, 262], F32, tag="aT")
                            nc.vector.tensor_copy(out=aT, in_=ta)
                            aS = ap2.tile([3, 262], F32, tag="aS")
                            nc.vector.tensor_copy(out=aS, in_=tsav[0:3, 0:262])
                            av = tsav[:, 262:393]
                            nc.tensor.matmul(
                                av, aT[:, 131:259], aT[:, 0:131], start=True, stop=False)
                            nc.tensor.matmul(
                                av, aS[0:3, 131:259], aS[0:3, 0:131], start=False, stop=True)
                            nc.scalar.copy(out=att[:, sb:sb + NN], in_=av)
                        emit_wo(s0 + ns)
                    emit_wo(BSH + 4)
                    assert wo_done[0] == len(wo_blocks) and em1[0] == 64
                    for cp_ in reversed(ctx_pools):
                        cp_.__exit__(None, None, None)
                bn_block(N1G[l], N1B[l], 2 * l)
                # FF block (ffb2 dropped: per-channel constant is absorbed by BN2)
                with tc.tile_pool(name=f"ffsb{l}", bufs=2) as fsb, \
                     tc.tile_pool(name=f"ffps{l}", bufs=2, space="PSUM") as fps:
                    em2 = [0]
                    for c0, w in _col_blocks(NPOS, 512):
                        f2ps = fps.tile([128, 512], F32, tag="f2ps")
                        for c in range(4):
                            f1ps = fps.tile([128, 512], F32, tag="f1ps")
                            nc.tensor.matmul(
                                f1ps[:, :w], mg[:, FW1[l] + c * 128:FW1[l] + c * 128 + 128],
                                hT[:, c0:c0 + w], start=True, stop=True)
                            rl = fsb.tile([128, 512], F32, tag="rl")
                            nc.scalar.activation(
                                out=rl[:, :w], in_=f1ps[:, :w], func=AF.Relu,
                                bias=mg[:, FB1[l] + c:FB1[l] + c + 1])
                            nc.tensor.matmul(
                                f2ps[:, :w], mg[:, FW2P[l] + c * 128:FW2P[l] + c * 128 + 128],
                                rl[:, :w], start=(c == 0), stop=(c == 3))
                        nc.vector.tensor_tensor(
                            out=hT[:, c0:c0 + w], in0=hT[:, c0:c0 + w],
                            in1=f2ps[:, :w], op=AluOpType.add)
                        emit_stats(2 * l + 1, em2, c0 + w)
                bn_block(N2G[l], N2B[l], 2 * l + 1)

            # ---------------- final head ----------------
            with tc.tile_pool(name="finsb", bufs=1) as fn, \
                 tc.tile_pool(name="finsb2", bufs=2) as fn2, \
                 tc.tile_pool(name="finps", bufs=2, space="PSUM") as fnp, \
                 tc.tile_pool(name="finps1", bufs=1, space="PSUM") as fnp1:
                vb = fn.tile([1, NPOS], BF16, tag="vb")
                nc.sync.dma_start(out=vb, in_=validbf[:])
                # emb = h * valid (zero invalid nodes), in place
                for c0, w in _col_blocks(NPOS, 512):
                    bps = fnp.tile([128, 512], F32, tag="bps")
                    nc.tensor.matmul(
                        bps[:, :w], ob[0:1, 0:128], vb[0:1, c0:c0 + w],
                        start=True, stop=True)
                    nc.vector.tensor_tensor(
                        out=hT[:, c0:c0 + w], in0=hT[:, c0:c0 + w],
                        in1=bps[:, :w], op=AluOpType.mult)
                gr = fn.tile([128, BSH], F32, tag="gr")
                for s0, ns in _sample_blocks(BSH, 8):
                    nc.vector.reduce_sum(
                        out=gr[:, s0:s0 + ns], in_=hT3[:, s0:s0 + ns, :], axis=AX.X)
                cps = fnp1.tile([128, 128], F32, tag="cps")
                nc.tensor.matmul(
                    cps[:], mg[:, WCTX:WCTX + 128], gr[:], start=True, stop=True)
                ctxT = fn.tile([128, BSH], F32, tag="ctxT")
                nc.vector.tensor_copy(out=ctxT, in_=cps)
                # comp = emb . (Wn @ Wctx^T @ graph): mega WNODE holds
                # (Wctx @ Wn^T) so U comes straight from graphT, not ctxT
                ups = fnp1.tile([128, BSH], F32, tag="ups")
                nc.tensor.matmul(
                    ups[:], mg[:, WNODE:WNODE + 128], gr[:], start=True, stop=True)
                uT = fn.tile([128, BSH], F32, tag="uT")
                nc.vector.tensor_copy(out=uT, in_=ups)
                for s0, ns in _sample_blocks(BSH, 3):
                    cp = fnp.tile([1, 393], F32, tag="cp")
                    for i in range(ns):
                        s = s0 + i
                        nc.tensor.matmul(
                            cp[0:1, i * NN:i * NN + NN], uT[:, s:s + 1],
                            hT[:, s * NN:s * NN + NN], start=True, stop=True)
                    stg = fn2.tile([1, 393], F32, tag="stg")
                    nc.vector.tensor_copy(out=stg[0:1, :ns * NN], in_=cp[0:1, :ns * NN])
                    src3 = stg[0:1, :ns * NN].rearrange("p (s n) -> p s n", n=NN)
                    nc.sync.dma_start(
                        out=OutComp[0:1, s0 * LH:(s0 + ns) * LH],
                        in_=src3[:, :, IH:IH + LH])
                tps = fnp1.tile([128, 128], F32, tag="tps")
                nc.tensor.transpose(tps[:], ctxT[:], ident)
                co = fn.tile([BSH, D], F32, tag="co")
                nc.vector.tensor_scalar_mul(
                    out=co, in0=tps[:], scalar1=mg[:, INVLEN:INVLEN + 1])
                nc.sync.dma_start(out=OutCtx[:], in_=co[:])
    return nc


def _prep_inputs(input, params):
    """Host-side preprocessing: shard + transpose + pack mega weight tile."""
    inp = np.ascontiguousarray(np.asarray(input, dtype=np.float32))
    p = params

    def f32(x):
        return np.asarray(x, dtype=np.float32)

    mega_common = np.zeros((128, MEGA_COLS), dtype=np.float32)
    mega_common[:, IDC:IDC + 128] = np.eye(128, dtype=np.float32)
    for l in range(2):
        lyr = p["layers"][l]
        mega_common[:, WQ[l]:WQ[l] + 128] = f32(lyr["wq"]) * np.float32(ISD)
        mega_common[:, WK[l]:WK[l] + 128] = f32(lyr["wk"])
        mega_common[:, WV[l]:WV[l] + 128] = f32(lyr["wv"]) @ f32(lyr["wo"])
        mega_common[:, WO[l]:WO[l] + 128] = f32(lyr["wo"])
        mega_common[:, FW1[l]:FW1[l] + 512] = f32(lyr["ff_w1"])
        for c in range(4):
            mega_common[:, FW2P[l] + c * 128:FW2P[l] + (c + 1) * 128] = \
                f32(lyr["ff_w2"])[c * 128:(c + 1) * 128, :]
            mega_common[:, FB1[l] + c] = f32(lyr["ff_b1"])[c * 128:(c + 1) * 128]
        mega_common[:, N1G[l]] = f32(lyr["n1_g"])
        mega_common[:, N1B[l]] = f32(lyr["n1_b"])
        mega_common[:, N2G[l]] = f32(lyr["n2_g"])
        mega_common[:, N2B[l]] = f32(lyr["n2_b"])
    mega_common[:, WCTX:WCTX + 128] = f32(p["w_ctx"])
    _wn = f32(p["w_node"])[:, :D] * np.float32(ISD)
    mega_common[:, WNODE:WNODE + 128] = f32(p["w_ctx"]) @ _wn.T
    mega_common[0:32, W2INT:W2INT + 128] = f32(p["int_w2"])
    mega_common[0:32, W2LEAF:W2LEAF + 128] = f32(p["leaf_w2"])
    mega_common[0:32, W2NEXT:W2NEXT + 128] = f32(p["next_w2"])
    mega_common[0:6, W1INT:W1INT + 32] = f32(p["int_w1"])
    mega_common[0:8, W1LEAF:W1LEAF + 32] = f32(p["leaf_w1"])
    mega_common[0:6, W1NEXT:W1NEXT + 32] = f32(p["next_w1"])
    mega_common[0:32, B1INT] = f32(p["int_b1"])
    mega_common[0:32, B1LEAF] = f32(p["leaf_b1"])
    mega_common[0:32, B1NEXT] = f32(p["next_b1"])
    mega_common[:, B2INT] = f32(p["int_b2"])
    mega_common[:, B2LEAF] = f32(p["leaf_b2"])
    mega_common[:, B2NEXT] = f32(p["next_b2"])

    valid = inp[..., 8]
    in_maps = []
    for c in range(NCORES):
        sl = slice(c * BSH, (c + 1) * BSH)
        xs = inp[sl]
        vs = valid[sl]
        mega = mega_common.copy()
        mega[:, INVLEN] = (1.0 / vs.sum(axis=1)).astype(np.float32)
        mega[:, LEAFM:LEAFM + LH] = vs[:, IH:IH + LH]
        xi = np.ascontiguousarray(
            xs[:, :IH, :6].reshape(BSH * IH, 6).T)
        xl = np.ascontiguousarray(
            xs[:, IH:IH + LH, :8].reshape(BSH * LH, 8).T)
        xn = np.ascontiguousarray(
            xs[:, IH + LH:, :6].reshape(BSH, 6).T)
        mask_add = np.where(vs < 0.5, np.float32(-1e9), np.float32(0.0))
        in_maps.append({
            "mega": mega,
            "xint": xi, "xleaf": xl, "xnext": xn,
            "masksbf": mask_add.astype(ml_dtypes.bfloat16),
            "validbf": (vs >= 0.5).astype(ml_dtypes.bfloat16).reshape(1, NPOS),
            "onesbf": np.ones((1, 128), dtype=ml_dtypes.bfloat16),
        })
    return in_maps


_NC_CACHE = None


def run_device(input, params, trace=False):
    global _NC_CACHE
    if _NC_CACHE is None:
        _NC_CACHE = build_bass()
        _NC_CACHE.finalize()
    nc = _NC_CACHE
    in_maps = _prep_inputs(input, params)
    res = run_bass_kernel_spmd(nc, in_maps, core_ids=list(range(NCORES)), trace=trace)
    comp = np.concatenate(
        [r["OutComp"].reshape(BSH, LH) for r in res.results], axis=0)
    ctx = np.concatenate([r["OutCtx"] for r in res.results], axis=0)
    return comp, ctx, res


def host_finish(comp, input):
    """comp: raw ctx.gK/sqrt(D) dot products for leaf nodes, [B, 50]."""
    valid = np.asarray(input, np.float32)[..., 8]
    inv_len = (1.0 / valid.sum(axis=1, dtype=np.float32)).astype(np.float32)
    logits = (np.tanh(comp * inv_len[:, None]) * np.float32(10.0)).astype(np.float32)
    p = np.exp(logits - logits.max(axis=1, keepdims=True)).astype(np.float32)
    p = (p / p.sum(axis=1, keepdims=True)).astype(np.float32)
    masked = p * valid[:, IH:IH + LH]
    probs = (masked / masked.sum(axis=1, keepdims=True)).astype(np.float32)
    return probs


def kernel(input, params):
    comp, ctx, _ = run_device(input, params)
    probs = host_finish(comp, input)
    # host finishing ops (cheap [B,50] reductions, fp32 to match reference)
    logp = np.log(np.clip(probs, np.float32(1e-38), None)).astype(np.float32)
    logp_safe = np.where(probs > 0, logp, np.float32(0.0))
    entropy = -(probs * logp_safe).sum(axis=-1)
    selected = np.argmax(probs, axis=-1).astype(np.int32)
    alp = np.take_along_axis(logp, selected[:, None].astype(np.int64), axis=1)
    return (alp.astype(np.float32), selected, entropy.astype(np.float32),
            ctx.astype(np.float32), probs.astype(np.float32))
